# revision 5
# baseline (speedup 1.0000x reference)
"""Top-1 gated MoE FFN (GLU experts) on 8 Trainium2 NeuronCores.

Expert-parallel: one expert per core. The host computes the (tiny) gate,
permutes tokens by their argmax expert (= the dispatch step of the
sharding strategy), each core runs its expert's FFN over its routed
tokens in bf16, and the host combines (scatter + per-token softmax
weight).

Shapes are hardcoded for the problem instance:
  x [8, 4096, 512], E=8 experts, D=512, F=2048.
"""

from contextlib import ExitStack

import numpy as np
import ml_dtypes

B, S, D, F, E = 8, 4096, 512, 2048, 8
N = B * S
P = 128            # SBUF partitions
TN = 512           # token supertile (matmul moving free dim)
ND = D // P        # 4 contraction chunks for x@w1 / x@w2
NF = F // P        # 16 F tiles

_BF16 = ml_dtypes.bfloat16

_kernel_cache: dict[int, object] = {}

# The activation used for the GLU gate. CoreSim doesn't implement Gelu, so
# debug harnesses may set this to "Tanh" to validate the kernel structure
# in simulation; hardware always uses Gelu.
ACT = "Gelu"


def _build_ffn(C: int):
    """Build + compile the per-core FFN kernel for capacity C tokens.

    Per-core compute (one expert):
      yT[:, j] = w3.T @ ( gelu(w1.T @ x_j + b1) * (w2.T @ x_j + b2) ) + b3
    for j in [0, C). x arrives pre-transposed as xT [D, C] bf16; output is
    yT [D, C] f32 (the host applies the per-token gate weight).
    """
    import concourse.tile as tile
    from concourse import bacc, mybir

    fp32 = mybir.dt.float32
    bf16 = mybir.dt.bfloat16
    AF = mybir.ActivationFunctionType
    ALU = mybir.AluOpType

    assert C % P == 0
    spans = []
    t0 = 0
    while t0 < C:
        spans.append((t0, min(TN, C - t0)))
        t0 += TN

    nc = bacc.Bacc("TRN2", target_bir_lowering=False, debug=False, num_devices=E)
    xT = nc.dram_tensor("xT", [D, C], bf16, kind="ExternalInput").ap()
    w1 = nc.dram_tensor("w1", [D, F], bf16, kind="ExternalInput").ap()
    w2 = nc.dram_tensor("w2", [D, F], bf16, kind="ExternalInput").ap()
    w3 = nc.dram_tensor("w3", [F, D], bf16, kind="ExternalInput").ap()
    b1 = nc.dram_tensor("b1", [F, 1], fp32, kind="ExternalInput").ap()
    b2 = nc.dram_tensor("b2", [F, 1], fp32, kind="ExternalInput").ap()
    b3 = nc.dram_tensor("b3", [D, 1], fp32, kind="ExternalInput").ap()
    yT = nc.dram_tensor("yT", [D, C], fp32, kind="ExternalOutput").ap()

    with tile.TileContext(nc) as tc, ExitStack() as ctx:
        wpool = ctx.enter_context(tc.tile_pool(name="weights", bufs=1))
        xpool = ctx.enter_context(tc.tile_pool(name="x", bufs=2))
        tpool = ctx.enter_context(tc.tile_pool(name="t1", bufs=2))
        gpool = ctx.enter_context(tc.tile_pool(name="g", bufs=3))
        dpool = ctx.enter_context(tc.tile_pool(name="drain", bufs=2))
        hpsum = ctx.enter_context(tc.tile_pool(name="h", bufs=2, space="PSUM"))
        opsum = ctx.enter_context(tc.tile_pool(name="acc", bufs=1, space="PSUM"))

        # Resident weights: w1/w2 as 4 chunks [128, F] (partition = D chunk),
        # w3 as 16 chunks [128, D] (partition = F chunk).
        w1_sb = []
        w2_sb = []
        for d in range(ND):
            t = wpool.tile([P, F], bf16, tag=f"w1_{d}", name=f"w1_{d}")
            nc.sync.dma_start(t[:], w1[d * P:(d + 1) * P, :])
            w1_sb.append(t)
            t = wpool.tile([P, F], bf16, tag=f"w2_{d}", name=f"w2_{d}")
            nc.sync.dma_start(t[:], w2[d * P:(d + 1) * P, :])
            w2_sb.append(t)
        w3_sb = []
        for f in range(NF):
            t = wpool.tile([P, D], bf16, tag=f"w3_{f}", name=f"w3_{f}")
            nc.sync.dma_start(t[:], w3[f * P:(f + 1) * P, :])
            w3_sb.append(t)
        # Biases, per-partition layout: [128, n_tiles]
        b1_sb = wpool.tile([P, NF], fp32, tag="b1")
        nc.sync.dma_start(b1_sb[:], b1.rearrange("(f p) o -> p (f o)", p=P))
        b2_sb = wpool.tile([P, NF], fp32, tag="b2")
        nc.sync.dma_start(b2_sb[:], b2.rearrange("(f p) o -> p (f o)", p=P))
        b3_sb = wpool.tile([P, ND], fp32, tag="b3")
        nc.sync.dma_start(b3_sb[:], b3.rearrange("(d p) o -> p (d o)", p=P))

        for (t0, tn) in spans:
            xt_sb = []
            for d in range(ND):
                t = xpool.tile([P, TN], bf16, tag=f"x{d}", name=f"x{d}")
                nc.sync.dma_start(t[:, :tn], xT[d * P:(d + 1) * P, t0:t0 + tn])
                xt_sb.append(t)
            po = [opsum.tile([P, TN], fp32, tag=f"po{dd}", name=f"po{dd}")
                  for dd in range(4)]

            # Software-pipelined by one f-step: the w3 accumulation for g[f-1]
            # is emitted while h1/h2 of f are being produced, so the PE never
            # waits on ACT/DVE.
            g_prev = None
            for f in range(NF):
                ph1 = hpsum.tile([P, TN], fp32, tag="ph1")
                for d in range(ND):
                    nc.tensor.matmul(
                        ph1[:, :tn], lhsT=w1_sb[d][:, f * P:(f + 1) * P],
                        rhs=xt_sb[d][:, :tn], start=(d == 0), stop=(d == ND - 1),
                    )
                ph2 = hpsum.tile([P, TN], fp32, tag="ph2")
                for d in range(ND):
                    nc.tensor.matmul(
                        ph2[:, :tn], lhsT=w2_sb[d][:, f * P:(f + 1) * P],
                        rhs=xt_sb[d][:, :tn], start=(d == 0), stop=(d == ND - 1),
                    )
                t1 = tpool.tile([P, TN], fp32, tag="t1")
                nc.scalar.activation(t1[:, :tn], ph1[:, :tn], getattr(AF, ACT),
                                     bias=b1_sb[:, f:f + 1])
                g = gpool.tile([P, TN], bf16, tag="g")
                nc.vector.scalar_tensor_tensor(
                    g[:, :tn], ph2[:, :tn], b2_sb[:, f:f + 1], t1[:, :tn],
                    op0=ALU.add, op1=ALU.mult,
                )
                if g_prev is not None:
                    fp, gp = g_prev
                    for dd in range(4):
                        nc.tensor.matmul(
                            po[dd][:, :tn], lhsT=w3_sb[fp][:, dd * P:(dd + 1) * P],
                            rhs=gp[:, :tn], start=(fp == 0), stop=False,
                        )
                g_prev = (f, g)
            fp, gp = g_prev
            for dd in range(4):
                nc.tensor.matmul(
                    po[dd][:, :tn], lhsT=w3_sb[fp][:, dd * P:(dd + 1) * P],
                    rhs=gp[:, :tn], start=False, stop=True,
                )
            for dd in range(4):
                osb = dpool.tile([P, TN], fp32, tag=f"osb{dd}", name=f"osb{dd}")
                nc.vector.tensor_scalar_add(osb[:, :tn], po[dd][:, :tn],
                                            b3_sb[:, dd:dd + 1])
                nc.sync.dma_start(yT[dd * P:(dd + 1) * P, t0:t0 + tn], osb[:, :tn])

    nc.compile()
    return nc


def _get_ffn(C: int):
    if C not in _kernel_cache:
        _kernel_cache[C] = _build_ffn(C)
    return _kernel_cache[C]


def _route(x, gate_w, gate_b):
    """Host gate: logits -> (top expert, gate weight, permutation, counts)."""
    xt = np.ascontiguousarray(np.asarray(x, dtype=np.float32).reshape(N, D))
    logits = xt @ np.asarray(gate_w, np.float32) + np.asarray(gate_b, np.float32)
    top = np.argmax(logits, axis=1)
    m = logits.max(axis=1, keepdims=True)
    ex = np.exp(logits - m)
    scale = ex[np.arange(N), top] / ex.sum(axis=1)
    order = np.argsort(top, kind="stable")
    counts = np.bincount(top, minlength=E)
    return xt, top, scale.astype(np.float32), order, counts


def prepare(x, gate_w, gate_b, w1, b1, w2, b2, w3, b3):
    """Everything up to the device call: routing + per-core input maps."""
    xt, top, scale, order, counts = _route(x, gate_w, gate_b)
    C = max(TN, int(-(-counts.max() // P)) * P)
    offs = np.zeros(E + 1, np.int64)
    offs[1:] = np.cumsum(counts)
    xp = xt[order]
    w1 = np.asarray(w1, np.float32)
    w2 = np.asarray(w2, np.float32)
    w3 = np.asarray(w3, np.float32)
    b1 = np.asarray(b1, np.float32)
    b2 = np.asarray(b2, np.float32)
    b3 = np.asarray(b3, np.float32)
    in_maps = []
    for ei in range(E):
        cnt = int(counts[ei])
        xTe = np.zeros((D, C), dtype=_BF16)
        if cnt:
            xTe[:, :cnt] = xp[offs[ei]:offs[ei + 1]].T.astype(_BF16)
        in_maps.append({
            "xT": xTe,
            "w1": np.ascontiguousarray(w1[ei]).astype(_BF16),
            "w2": np.ascontiguousarray(w2[ei]).astype(_BF16),
            "w3": np.ascontiguousarray(w3[ei]).astype(_BF16),
            "b1": np.ascontiguousarray(b1[ei]).reshape(F, 1),
            "b2": np.ascontiguousarray(b2[ei]).reshape(F, 1),
            "b3": np.ascontiguousarray(b3[ei]).reshape(D, 1),
        })
    return in_maps, C, order, offs, counts, scale


def combine(results, order, offs, counts, scale):
    """Scatter per-expert outputs back and apply the gate weight."""
    out = np.zeros((N, D), np.float32)
    for ei in range(E):
        cnt = int(counts[ei])
        if not cnt:
            continue
        yTe = results[ei]["yT"]                      # [D, C] f32
        idx = order[offs[ei]:offs[ei + 1]]
        out[idx] = yTe[:, :cnt].T * scale[idx][:, None]
    return out.reshape(B, S, D)


def kernel(x, gate_w, gate_b, w1, b1, w2, b2, w3, b3):
    from concourse.bass_utils import run_bass_kernel_spmd

    in_maps, C, order, offs, counts, scale = prepare(
        x, gate_w, gate_b, w1, b1, w2, b2, w3, b3)
    nc = _get_ffn(C)
    res = run_bass_kernel_spmd(nc, in_maps, core_ids=list(range(E)))
    out = combine(res.results, order, offs, counts, scale)
    aux_loss = np.asarray(0.0, dtype=np.float32)
    return out, aux_loss
